# revision 6
# baseline (speedup 1.0000x reference)
"""Fused cross-entropy label-propagation kernel for Trainium2 (8 cores), v2.

Computation (per batch b):
  sim   = ref_flat(b) @ tgt_flat(b)          # [12288, 4096]
  prob  = softmax(sim, axis=0)               # over ref pixels
  pred  = lab_flat(b) @ prob                 # [16, 4096]
  loss  = mean(-log(pred[label] + eps))

Sharding: batch b = core // 4, target-pixel columns split 4-way per batch.

v2 changes vs v1:
- sim matmul in fp8 e4m3 (inputs scaled x16) using DoubleRow perf mode:
  one matmul contracts both 128-halves of F=256 -> half the PE passes.
  End-to-end loss error from fp8 sim quantization alone is ~1.6e-3.
- exp on ScalarE with bf16 output. (An optional DVE Schraudolph fast-exp
  path exists below — u16(x * 2^7/ln2 + (127*128 - C)) bitcast to bf16,
  with u16 saturation mapping underflow to +0.0 and overflow to NaN — but
  it is disabled: measured ~620ns/tile on DVE vs ~330ns/tile on ScalarE,
  so offloading exp tiles lost time in every tested split.)
- label matmul in bf16 (labels and p in bf16).

The constant shift replaces the per-column max (see v1 docstring): host
validates each column (den finite, 1e-20 < den < 1e38) and reruns with
shifted windows to patch any column outside the exp window.
"""

import numpy as np

B, NREF, F, H, W, D = 2, 3, 256, 64, 64, 16
T = H * W                     # 4096 target pixels per batch
N = NREF * T                  # 12288 ref pixels per batch
NCORES = 8
T_LOC = B * T // NCORES       # 1024 columns per core
NT = N // 128                 # 96 ref-row tiles
GK = 8                        # ref k-tiles per DMA group
SHIFT0 = 136.0                # subtracted from sim before exp
SC = 16.0                     # fp8 quantization scale for ref/tgt
EPS = 1e-14
DVE_MOD, DVE_RES = 3, 2       # k % DVE_MOD >= DVE_RES -> DVE fast-exp path
LAG = 6                       # label matmul trails exp by LAG tiles: PE
# executes its queue in order, so the label matmul for tile k blocks PE on
# exp(k); a deep lag keeps slow exp tiles (DVE ~620ns vs ScalarE ~330ns)
# from stalling sim production.

M16 = float(2.0 ** 7 / np.log(2.0))
C16 = 5.6

_CACHE = {}
LAST_RESULTS = None


def _build_program(reps=1, shift=SHIFT0):
    key = ("nc", reps, shift)
    if key in _CACHE:
        return _CACHE[key]

    import concourse.bacc as bacc
    import concourse.tile as tile
    import concourse.mybir as mybir

    f32 = mybir.dt.float32
    bf16 = mybir.dt.bfloat16
    u16 = mybir.dt.uint16
    f8 = mybir.dt.float8e4

    nc = bacc.Bacc("TRN2", target_bir_lowering=False, debug=False,
                   num_devices=NCORES)

    ref_d = nc.dram_tensor("ref", [128, NT, 2, 128], f8, kind="ExternalInput")
    tgt_d = nc.dram_tensor("tgt", [128, 2, T_LOC], f8, kind="ExternalInput")
    lab_d = nc.dram_tensor("lab", [128, NT * (D + 1)], bf16,
                           kind="ExternalInput")
    shv_d = nc.dram_tensor("shv", [128, 1], f32, kind="ExternalInput")
    out_d = nc.dram_tensor("out", [D + 1, T_LOC], f32, kind="ExternalOutput")

    # DVE affine constants (fold fp8 scale + shift into the Schraudolph line)
    a_dve = M16 / (SC * SC)
    b_dve = float(127.0 * 128.0 - C16 - M16 * shift)

    with tile.TileContext(nc) as tc:
        with (
            tc.tile_pool(name="refs", bufs=1) as refs,
            tc.tile_pool(name="small", bufs=1) as small,
            tc.tile_pool(name="ppool", bufs=6) as ppool,
            tc.tile_pool(name="zpool", bufs=4) as zpool,
            tc.tile_pool(name="simpool", bufs=3, space="PSUM") as simpool,
            tc.tile_pool(name="predpool", bufs=1, space="PSUM") as predpool,
        ):
            tgt_sb = small.tile([128, 2, T_LOC], f8, tag="tgt")
            nc.sync.dma_start(out=tgt_sb, in_=tgt_d[:])
            bias_sb = small.tile([128, 1], f32, tag="bias")
            nc.sync.dma_start(out=bias_sb, in_=shv_d[:])

            ref_sb = []
            lab_sb = None
            for g in range(NT // GK):
                rt = refs.tile([128, GK, 2, 128], f8, tag=f"ref{g}")
                nc.sync.dma_start(out=rt, in_=ref_d[:, g * GK:(g + 1) * GK])
                ref_sb.append(rt)
                if lab_sb is None:
                    lab_sb = small.tile([128, NT * (D + 1)], bf16, tag="lab")
                    nc.sync.dma_start(out=lab_sb, in_=lab_d[:])

            for rep in range(reps):
                pred = predpool.tile([D + 1, T_LOC], f32, tag="pred")

                def label_mm(k, p, pred=pred):
                    for cc in range(2):
                        nc.tensor.matmul(
                            pred[:, cc * 512:(cc + 1) * 512],
                            lhsT=lab_sb[:, k * (D + 1):(k + 1) * (D + 1)],
                            rhs=p[:, cc * 512:(cc + 1) * 512],
                            start=(k == 0), stop=(k == NT - 1),
                        )

                pending = []
                for k in range(NT):
                    g, j = divmod(k, GK)
                    sim = simpool.tile([128, T_LOC], f32, tag="sim")
                    for cc in range(2):
                        nc.tensor.matmul(
                            sim[:, cc * 512:(cc + 1) * 512],
                            lhsT=ref_sb[g][:, j],
                            rhs=tgt_sb[:, :, cc * 512:(cc + 1) * 512],
                            start=True, stop=True,
                            perf_mode=mybir.MatmulPerfMode.DoubleRow,
                        )
                    if k % DVE_MOD >= DVE_RES:
                        z = zpool.tile([128, T_LOC], u16, tag="z")
                        nc.vector.tensor_scalar(
                            z, sim, a_dve, b_dve,
                            mybir.AluOpType.mult, mybir.AluOpType.add)
                        p = z.bitcast(bf16)
                    else:
                        p = ppool.tile([128, T_LOC], bf16, tag="p")
                        nc.scalar.activation(
                            out=p, in_=sim,
                            func=mybir.ActivationFunctionType.Exp,
                            bias=bias_sb[:], scale=1.0 / (SC * SC))
                    pending.append((k, p))
                    if len(pending) > LAG:
                        label_mm(*pending.pop(0))
                while pending:
                    label_mm(*pending.pop(0))

                po = small.tile([D + 1, T_LOC], f32, tag="po")
                nc.vector.tensor_copy(po, pred)
                nc.sync.dma_start(out=out_d[:], in_=po)

    nc.compile()
    _CACHE[key] = nc
    return nc


def _prep_inputs(ref, target, ref_label):
    """Per-batch host-side relayouts shared by the 4 cores of each batch."""
    import ml_dtypes
    per_b = []
    for b in range(B):
        # ref_flat [N, F]; lhsT tile k: ref8[p, k, i, m] = ref_flat[k*128+m,
        # i*128+p] * SC
        rf = ref[b].transpose(0, 2, 3, 1).reshape(N, F)
        r4 = rf.reshape(NT, 128, 2, 128)          # [k, m, i, p]
        ref8 = np.ascontiguousarray(
            r4.transpose(3, 0, 2, 1) * SC).astype(ml_dtypes.float8_e4m3)
        # tgt8[p, i, t] = tgt_flat[i*128+p, t] * SC   (full T; shard later)
        tf = target[b].reshape(2, 128, T)          # [i, p, t]
        tgt8 = np.ascontiguousarray(
            tf.transpose(1, 0, 2) * SC).astype(ml_dtypes.float8_e4m3)
        # labels: n = (r, h, w) major -> [12288, 16], append ones -> [.., 17]
        labn = ref_label[b].transpose(0, 2, 3, 1).reshape(N, D)
        labo = np.concatenate(
            [labn, np.ones((N, 1), np.float32)], axis=1)
        labsb = np.ascontiguousarray(
            labo.reshape(NT, 128, D + 1).transpose(1, 0, 2).reshape(128, -1)
        ).astype(ml_dtypes.bfloat16)
        per_b.append((ref8, tgt8, labsb))
    return per_b


def _make_in_maps(per_b, shift):
    shv = np.full((128, 1), -shift, np.float32)
    in_maps = []
    for core in range(NCORES):
        b, s = divmod(core, NCORES // B)
        ref8, tgt8, labsb = per_b[b]
        in_maps.append({
            "ref": ref8,
            "tgt": np.ascontiguousarray(tgt8[:, :, s * T_LOC:(s + 1) * T_LOC]),
            "lab": labsb,
            "shv": shv,
        })
    return in_maps


def _run_cores(per_b, shift):
    """One SPMD run with the given softmax shift; returns per-batch [17, 4096]."""
    global LAST_RESULTS
    from concourse.bass_utils import run_bass_kernel_spmd

    nc = _build_program(shift=shift)
    LAST_RESULTS = run_bass_kernel_spmd(nc, _make_in_maps(per_b, shift),
                                        list(range(NCORES)))
    outs = LAST_RESULTS.results
    return [
        np.concatenate(
            [outs[b * (NCORES // B) + s]["out"] for s in range(NCORES // B)],
            axis=1).astype(np.float64)
        for b in range(B)
    ]


def _bad_cols(raw):
    """Columns whose exp window misbehaved for the used shift."""
    with np.errstate(all="ignore"):
        den, num = raw[D], raw[:D]
        return (~np.isfinite(den) | (den <= 1e-20) | (den >= 1e38)
                | ~np.isfinite(num).all(axis=0))


def kernel(ref, target, ref_label, target_label):
    ref = np.asarray(ref, np.float32)
    target = np.asarray(target, np.float32)
    ref_label = np.asarray(ref_label, np.float32)
    labels = np.asarray(target_label).astype(np.int64)

    per_b = _prep_inputs(ref, target, ref_label)
    raws = _run_cores(per_b, SHIFT0)

    # Rescue any columns outside the exp window with shifted reruns (a no-op
    # for data resembling the reference distribution).
    bad = [_bad_cols(r) for r in raws]
    for delta in (60.0, -60.0, 120.0, -120.0):
        if not any(bm.any() for bm in bad):
            break
        raws2 = _run_cores(per_b, SHIFT0 + delta)
        for b in range(B):
            fixable = bad[b] & ~_bad_cols(raws2[b])
            raws[b][:, fixable] = raws2[b][:, fixable]
            bad[b] &= ~fixable

    nll_sum = 0.0
    with np.errstate(all="ignore"):
        for b in range(B):
            pred = raws[b][:D] / raws[b][D]                  # [16, 4096]
            logp = np.log(pred + EPS)
            idx = labels[b].reshape(T)
            nll_sum += -logp[idx, np.arange(T)].sum()
    loss = nll_sum / (B * T)
    return np.asarray(loss, dtype=np.float32)


# revision 8
# speedup vs baseline: 6.5633x; 6.5633x over previous
"""Fused cross-entropy label-propagation kernel for Trainium2 (8 cores), v2.

Computation (per batch b):
  sim   = ref_flat(b) @ tgt_flat(b)          # [12288, 4096]
  prob  = softmax(sim, axis=0)               # over ref pixels
  pred  = lab_flat(b) @ prob                 # [16, 4096]
  loss  = mean(-log(pred[label] + eps))

Sharding: batch b = core // 4, target-pixel columns split 4-way per batch.

v2 changes vs v1:
- sim matmul in fp8 e4m3 (inputs scaled x16) using DoubleRow perf mode:
  one matmul contracts both 128-halves of F=256 -> half the PE passes.
  End-to-end loss error from fp8 sim quantization alone is ~1.6e-3.
- exp on ScalarE with bf16 output. (An optional DVE Schraudolph fast-exp
  path exists below — u16(x * 2^7/ln2 + (127*128 - C)) bitcast to bf16,
  with u16 saturation mapping underflow to +0.0 and overflow to NaN — but
  it is disabled: measured ~620ns/tile on DVE vs ~330ns/tile on ScalarE,
  so offloading exp tiles lost time in every tested split.)
- label matmul in bf16 (labels and p in bf16).

The constant shift replaces the per-column max (see v1 docstring): host
validates each column (den finite, 1e-20 < den < 1e38) and reruns with
shifted windows to patch any column outside the exp window.
"""

import numpy as np

B, NREF, F, H, W, D = 2, 3, 256, 64, 64, 16
T = H * W                     # 4096 target pixels per batch
N = NREF * T                  # 12288 ref pixels per batch
NCORES = 8
T_LOC = B * T // NCORES       # 1024 columns per core
NT = N // 128                 # 96 ref-row tiles
GK = 8                        # ref k-tiles per DMA group
SHIFT0 = 136.0                # subtracted from sim before exp
SC = 16.0                     # fp8 quantization scale for ref/tgt
EPS = 1e-14
DVE_MOD, DVE_RES = 3, 2       # k % DVE_MOD >= DVE_RES -> DVE fast-exp path
LAG = 6                       # label matmul trails exp by LAG tiles: PE
# executes its queue in order, so the label matmul for tile k blocks PE on
# exp(k); a deep lag keeps slow exp tiles (DVE ~620ns vs ScalarE ~330ns)
# from stalling sim production.

M16 = float(2.0 ** 7 / np.log(2.0))
C16 = 5.6

_CACHE = {}
LAST_RESULTS = None


def _build_program(reps=1, shift=SHIFT0):
    key = ("nc", reps, shift)
    if key in _CACHE:
        return _CACHE[key]

    import concourse.bacc as bacc
    import concourse.tile as tile
    import concourse.mybir as mybir

    f32 = mybir.dt.float32
    bf16 = mybir.dt.bfloat16
    u16 = mybir.dt.uint16
    f8 = mybir.dt.float8e4

    nc = bacc.Bacc("TRN2", target_bir_lowering=False, debug=False,
                   num_devices=NCORES)

    ref_d = nc.dram_tensor("ref", [128, NT, 2, 128], f8, kind="ExternalInput")
    tgt_d = nc.dram_tensor("tgt", [128, 2, T_LOC], f8, kind="ExternalInput")
    lab_d = nc.dram_tensor("lab", [128, NT * (D + 1)], bf16,
                           kind="ExternalInput")
    shv_d = nc.dram_tensor("shv", [128, 1], f32, kind="ExternalInput")
    out_d = nc.dram_tensor("out", [D + 1, T_LOC], f32, kind="ExternalOutput")

    # DVE affine constants (fold fp8 scale + shift into the Schraudolph line)
    a_dve = M16 / (SC * SC)
    b_dve = float(127.0 * 128.0 - C16 - M16 * shift)

    with tile.TileContext(nc) as tc:
        with (
            tc.tile_pool(name="refs", bufs=1) as refs,
            tc.tile_pool(name="small", bufs=1) as small,
            tc.tile_pool(name="ppool", bufs=6) as ppool,
            tc.tile_pool(name="zpool", bufs=4) as zpool,
            tc.tile_pool(name="simpool", bufs=3, space="PSUM") as simpool,
            tc.tile_pool(name="predpool", bufs=1, space="PSUM") as predpool,
        ):
            tgt_sb = small.tile([128, 2, T_LOC], f8, tag="tgt")
            nc.sync.dma_start(out=tgt_sb, in_=tgt_d[:])
            bias_sb = small.tile([128, 1], f32, tag="bias")
            nc.sync.dma_start(out=bias_sb, in_=shv_d[:])

            ref_sb = []
            lab_sb = None
            for g in range(NT // GK):
                rt = refs.tile([128, GK, 2, 128], f8, tag=f"ref{g}")
                nc.sync.dma_start(out=rt, in_=ref_d[:, g * GK:(g + 1) * GK])
                ref_sb.append(rt)
                if lab_sb is None:
                    lab_sb = small.tile([128, NT * (D + 1)], bf16, tag="lab")
                    nc.sync.dma_start(out=lab_sb, in_=lab_d[:])

            for rep in range(reps):
                pred = predpool.tile([D + 1, T_LOC], f32, tag="pred")

                def label_mm(k, p, pred=pred):
                    for cc in range(2):
                        nc.tensor.matmul(
                            pred[:, cc * 512:(cc + 1) * 512],
                            lhsT=lab_sb[:, k * (D + 1):(k + 1) * (D + 1)],
                            rhs=p[:, cc * 512:(cc + 1) * 512],
                            start=(k == 0), stop=(k == NT - 1),
                        )

                pending = []
                for k in range(NT):
                    g, j = divmod(k, GK)
                    sim = simpool.tile([128, T_LOC], f32, tag="sim")
                    for cc in range(2):
                        nc.tensor.matmul(
                            sim[:, cc * 512:(cc + 1) * 512],
                            lhsT=ref_sb[g][:, j],
                            rhs=tgt_sb[:, :, cc * 512:(cc + 1) * 512],
                            start=True, stop=True,
                            perf_mode=mybir.MatmulPerfMode.DoubleRow,
                        )
                    if k % DVE_MOD >= DVE_RES:
                        z = zpool.tile([128, T_LOC], u16, tag="z")
                        nc.vector.tensor_scalar(
                            z, sim, a_dve, b_dve,
                            mybir.AluOpType.mult, mybir.AluOpType.add)
                        p = z.bitcast(bf16)
                    else:
                        p = ppool.tile([128, T_LOC], bf16, tag="p")
                        nc.scalar.activation(
                            out=p, in_=sim,
                            func=mybir.ActivationFunctionType.Exp,
                            bias=bias_sb[:], scale=1.0 / (SC * SC))
                    pending.append((k, p))
                    if len(pending) > LAG:
                        label_mm(*pending.pop(0))
                while pending:
                    label_mm(*pending.pop(0))

                po = small.tile([D + 1, T_LOC], f32, tag="po")
                nc.vector.tensor_copy(po, pred)
                nc.sync.dma_start(out=out_d[:], in_=po)

    nc.compile()
    _CACHE[key] = nc
    return nc


def _prep_inputs(ref, target, ref_label):
    """Per-batch host-side relayouts shared by the 4 cores of each batch."""
    import ml_dtypes
    per_b = []
    for b in range(B):
        # ref_flat [N, F]; lhsT tile k: ref8[p, k, i, m] = ref_flat[k*128+m,
        # i*128+p] * SC
        rf = ref[b].transpose(0, 2, 3, 1).reshape(N, F)
        r4 = rf.reshape(NT, 128, 2, 128)          # [k, m, i, p]
        ref8 = np.ascontiguousarray(
            r4.transpose(3, 0, 2, 1) * SC).astype(ml_dtypes.float8_e4m3)
        # tgt8[p, i, t] = tgt_flat[i*128+p, t] * SC   (full T; shard later)
        tf = target[b].reshape(2, 128, T)          # [i, p, t]
        tgt8 = np.ascontiguousarray(
            tf.transpose(1, 0, 2) * SC).astype(ml_dtypes.float8_e4m3)
        # labels: n = (r, h, w) major -> [12288, 16], append ones -> [.., 17]
        labn = ref_label[b].transpose(0, 2, 3, 1).reshape(N, D)
        labo = np.concatenate(
            [labn, np.ones((N, 1), np.float32)], axis=1)
        labsb = np.ascontiguousarray(
            labo.reshape(NT, 128, D + 1).transpose(1, 0, 2).reshape(128, -1)
        ).astype(ml_dtypes.bfloat16)
        per_b.append((ref8, tgt8, labsb))
    return per_b


def _make_in_maps(per_b, shift):
    shv = np.full((128, 1), -shift, np.float32)
    in_maps = []
    for core in range(NCORES):
        b, s = divmod(core, NCORES // B)
        ref8, tgt8, labsb = per_b[b]
        in_maps.append({
            "ref": ref8,
            "tgt": np.ascontiguousarray(tgt8[:, :, s * T_LOC:(s + 1) * T_LOC]),
            "lab": labsb,
            "shv": shv,
        })
    return in_maps


def _run_cores(per_b, shift):
    """One SPMD run with the given softmax shift; returns per-batch [17, 4096]."""
    global LAST_RESULTS
    from concourse.bass_utils import run_bass_kernel_spmd

    nc = _build_program(shift=shift)
    LAST_RESULTS = run_bass_kernel_spmd(nc, _make_in_maps(per_b, shift),
                                        list(range(NCORES)))
    outs = LAST_RESULTS.results
    return [
        np.concatenate(
            [outs[b * (NCORES // B) + s]["out"] for s in range(NCORES // B)],
            axis=1).astype(np.float64)
        for b in range(B)
    ]


def _bad_cols(raw):
    """Columns whose exp window misbehaved for the used shift."""
    with np.errstate(all="ignore"):
        den, num = raw[D], raw[:D]
        return (~np.isfinite(den) | (den <= 1e-20) | (den >= 1e38)
                | ~np.isfinite(num).all(axis=0))


def kernel(ref, target, ref_label, target_label):
    ref = np.asarray(ref, np.float32)
    target = np.asarray(target, np.float32)
    ref_label = np.asarray(ref_label, np.float32)
    labels = np.asarray(target_label).astype(np.int64)

    per_b = _prep_inputs(ref, target, ref_label)
    raws = _run_cores(per_b, SHIFT0)

    # Rescue any columns outside the exp window with shifted reruns (a no-op
    # for data resembling the reference distribution).
    bad = [_bad_cols(r) for r in raws]
    for delta in (60.0, -60.0, 120.0, -120.0):
        if not any(bm.any() for bm in bad):
            break
        raws2 = _run_cores(per_b, SHIFT0 + delta)
        for b in range(B):
            fixable = bad[b] & ~_bad_cols(raws2[b])
            raws[b][:, fixable] = raws2[b][:, fixable]
            bad[b] &= ~fixable

    nll_sum = 0.0
    with np.errstate(all="ignore"):
        for b in range(B):
            pred = raws[b][:D] / raws[b][D]                  # [16, 4096]
            logp = np.log(pred + EPS)
            idx = labels[b].reshape(T)
            nll_sum += -logp[idx, np.arange(T)].sum()
    loss = nll_sum / (B * T)
    return np.asarray(loss, dtype=np.float32)


# revision 9
# speedup vs baseline: 17.0641x; 2.5999x over previous
"""Fused cross-entropy label-propagation kernel for Trainium2 (8 cores), v2.

Computation (per batch b):
  sim   = ref_flat(b) @ tgt_flat(b)          # [12288, 4096]
  prob  = softmax(sim, axis=0)               # over ref pixels
  pred  = lab_flat(b) @ prob                 # [16, 4096]
  loss  = mean(-log(pred[label] + eps))

Sharding: batch b = core // 4, target-pixel columns split 4-way per batch.

v2 changes vs v1:
- sim matmul in fp8 e4m3 (inputs scaled x16) using DoubleRow perf mode:
  one matmul contracts both 128-halves of F=256 -> half the PE passes.
  End-to-end loss error from fp8 sim quantization alone is ~1.6e-3.
- exp on ScalarE with bf16 output. (An optional DVE Schraudolph fast-exp
  path exists below — u16(x * 2^7/ln2 + (127*128 - C)) bitcast to bf16,
  with u16 saturation mapping underflow to +0.0 and overflow to NaN — but
  it is disabled: measured ~620ns/tile on DVE vs ~330ns/tile on ScalarE,
  so offloading exp tiles lost time in every tested split.)
- label matmul in bf16 (labels and p in bf16).

The constant shift replaces the per-column max (see v1 docstring): host
validates each column (den finite, 1e-20 < den < 1e38) and reruns with
shifted windows to patch any column outside the exp window.
"""

import numpy as np

B, NREF, F, H, W, D = 2, 3, 256, 64, 64, 16
T = H * W                     # 4096 target pixels per batch
N = NREF * T                  # 12288 ref pixels per batch
NCORES = 8
T_LOC = B * T // NCORES       # 1024 columns per core
NT = N // 128                 # 96 ref-row tiles
GK = 8                        # ref k-tiles per DMA group
SHIFT0 = 136.0                # subtracted from sim before exp
SC = 16.0                     # fp8 quantization scale for ref/tgt
EPS = 1e-14
DVE_MOD, DVE_RES = 4, 3       # k % DVE_MOD >= DVE_RES -> DVE fast-exp path
LAG = 6                       # label matmul trails exp by LAG tiles: PE
# executes its queue in order, so the label matmul for tile k blocks PE on
# exp(k); a deep lag keeps slow exp tiles (DVE ~620ns vs ScalarE ~330ns)
# from stalling sim production.

M16 = float(2.0 ** 7 / np.log(2.0))
C16 = 5.6

_CACHE = {}
LAST_RESULTS = None


def _build_program(reps=1, shift=SHIFT0):
    key = ("nc", reps, shift)
    if key in _CACHE:
        return _CACHE[key]

    import concourse.bacc as bacc
    import concourse.tile as tile
    import concourse.mybir as mybir

    f32 = mybir.dt.float32
    bf16 = mybir.dt.bfloat16
    u16 = mybir.dt.uint16
    f8 = mybir.dt.float8e4

    nc = bacc.Bacc("TRN2", target_bir_lowering=False, debug=False,
                   num_devices=NCORES)

    ref_d = nc.dram_tensor("ref", [128, NT, 2, 128], f8, kind="ExternalInput")
    tgt_d = nc.dram_tensor("tgt", [128, 2, T_LOC], f8, kind="ExternalInput")
    lab_d = nc.dram_tensor("lab", [128, NT * (D + 1)], bf16,
                           kind="ExternalInput")
    shv_d = nc.dram_tensor("shv", [128, 1], f32, kind="ExternalInput")
    out_d = nc.dram_tensor("out", [D + 1, T_LOC], f32, kind="ExternalOutput")

    # DVE affine constants (fold fp8 scale + shift into the Schraudolph line)
    a_dve = M16 / (SC * SC)
    b_dve = float(127.0 * 128.0 - C16 - M16 * shift)

    with tile.TileContext(nc) as tc:
        with (
            tc.tile_pool(name="refs", bufs=1) as refs,
            tc.tile_pool(name="small", bufs=1) as small,
            tc.tile_pool(name="ppool", bufs=6) as ppool,
            tc.tile_pool(name="zpool", bufs=4) as zpool,
            tc.tile_pool(name="simpool", bufs=3, space="PSUM") as simpool,
            tc.tile_pool(name="predpool", bufs=1, space="PSUM") as predpool,
        ):
            tgt_sb = small.tile([128, 2, T_LOC], f8, tag="tgt")
            nc.sync.dma_start(out=tgt_sb, in_=tgt_d[:])
            bias_sb = small.tile([128, 1], f32, tag="bias")
            nc.sync.dma_start(out=bias_sb, in_=shv_d[:])

            ref_sb = []
            lab_sb = None
            for g in range(NT // GK):
                rt = refs.tile([128, GK, 2, 128], f8, tag=f"ref{g}")
                nc.sync.dma_start(out=rt, in_=ref_d[:, g * GK:(g + 1) * GK])
                ref_sb.append(rt)
                if lab_sb is None:
                    lab_sb = small.tile([128, NT * (D + 1)], bf16, tag="lab")
                    nc.sync.dma_start(out=lab_sb, in_=lab_d[:])

            for rep in range(reps):
                pred = predpool.tile([D + 1, T_LOC], f32, tag="pred")

                def label_mm(k, p, pred=pred):
                    for cc in range(2):
                        nc.tensor.matmul(
                            pred[:, cc * 512:(cc + 1) * 512],
                            lhsT=lab_sb[:, k * (D + 1):(k + 1) * (D + 1)],
                            rhs=p[:, cc * 512:(cc + 1) * 512],
                            start=(k == 0), stop=(k == NT - 1),
                        )

                pending = []
                for k in range(NT):
                    g, j = divmod(k, GK)
                    sim = simpool.tile([128, T_LOC], f32, tag="sim")
                    for cc in range(2):
                        nc.tensor.matmul(
                            sim[:, cc * 512:(cc + 1) * 512],
                            lhsT=ref_sb[g][:, j],
                            rhs=tgt_sb[:, :, cc * 512:(cc + 1) * 512],
                            start=True, stop=True,
                            perf_mode=mybir.MatmulPerfMode.DoubleRow,
                        )
                    if k % DVE_MOD >= DVE_RES:
                        z = zpool.tile([128, T_LOC], u16, tag="z")
                        nc.vector.tensor_scalar(
                            z, sim, a_dve, b_dve,
                            mybir.AluOpType.mult, mybir.AluOpType.add)
                        p = z.bitcast(bf16)
                    else:
                        p = ppool.tile([128, T_LOC], bf16, tag="p")
                        nc.scalar.activation(
                            out=p, in_=sim,
                            func=mybir.ActivationFunctionType.Exp,
                            bias=bias_sb[:], scale=1.0 / (SC * SC))
                    pending.append((k, p))
                    if len(pending) > LAG:
                        label_mm(*pending.pop(0))
                while pending:
                    label_mm(*pending.pop(0))

                po = small.tile([D + 1, T_LOC], f32, tag="po")
                nc.vector.tensor_copy(po, pred)
                nc.sync.dma_start(out=out_d[:], in_=po)

    nc.compile()
    _CACHE[key] = nc
    return nc


def _prep_inputs(ref, target, ref_label):
    """Per-batch host-side relayouts shared by the 4 cores of each batch."""
    import ml_dtypes
    per_b = []
    for b in range(B):
        # ref_flat [N, F]; lhsT tile k: ref8[p, k, i, m] = ref_flat[k*128+m,
        # i*128+p] * SC
        rf = ref[b].transpose(0, 2, 3, 1).reshape(N, F)
        r4 = rf.reshape(NT, 128, 2, 128)          # [k, m, i, p]
        ref8 = np.ascontiguousarray(
            r4.transpose(3, 0, 2, 1) * SC).astype(ml_dtypes.float8_e4m3)
        # tgt8[p, i, t] = tgt_flat[i*128+p, t] * SC   (full T; shard later)
        tf = target[b].reshape(2, 128, T)          # [i, p, t]
        tgt8 = np.ascontiguousarray(
            tf.transpose(1, 0, 2) * SC).astype(ml_dtypes.float8_e4m3)
        # labels: n = (r, h, w) major -> [12288, 16], append ones -> [.., 17]
        labn = ref_label[b].transpose(0, 2, 3, 1).reshape(N, D)
        labo = np.concatenate(
            [labn, np.ones((N, 1), np.float32)], axis=1)
        labsb = np.ascontiguousarray(
            labo.reshape(NT, 128, D + 1).transpose(1, 0, 2).reshape(128, -1)
        ).astype(ml_dtypes.bfloat16)
        per_b.append((ref8, tgt8, labsb))
    return per_b


def _make_in_maps(per_b, shift):
    shv = np.full((128, 1), -shift, np.float32)
    in_maps = []
    for core in range(NCORES):
        b, s = divmod(core, NCORES // B)
        ref8, tgt8, labsb = per_b[b]
        in_maps.append({
            "ref": ref8,
            "tgt": np.ascontiguousarray(tgt8[:, :, s * T_LOC:(s + 1) * T_LOC]),
            "lab": labsb,
            "shv": shv,
        })
    return in_maps


def _run_cores(per_b, shift):
    """One SPMD run with the given softmax shift; returns per-batch [17, 4096]."""
    global LAST_RESULTS
    from concourse.bass_utils import run_bass_kernel_spmd

    nc = _build_program(shift=shift)
    LAST_RESULTS = run_bass_kernel_spmd(nc, _make_in_maps(per_b, shift),
                                        list(range(NCORES)))
    outs = LAST_RESULTS.results
    return [
        np.concatenate(
            [outs[b * (NCORES // B) + s]["out"] for s in range(NCORES // B)],
            axis=1).astype(np.float64)
        for b in range(B)
    ]


def _bad_cols(raw):
    """Columns whose exp window misbehaved for the used shift."""
    with np.errstate(all="ignore"):
        den, num = raw[D], raw[:D]
        return (~np.isfinite(den) | (den <= 1e-20) | (den >= 1e38)
                | ~np.isfinite(num).all(axis=0))


def kernel(ref, target, ref_label, target_label):
    ref = np.asarray(ref, np.float32)
    target = np.asarray(target, np.float32)
    ref_label = np.asarray(ref_label, np.float32)
    labels = np.asarray(target_label).astype(np.int64)

    per_b = _prep_inputs(ref, target, ref_label)
    raws = _run_cores(per_b, SHIFT0)

    # Rescue any columns outside the exp window with shifted reruns (a no-op
    # for data resembling the reference distribution).
    bad = [_bad_cols(r) for r in raws]
    for delta in (60.0, -60.0, 120.0, -120.0):
        if not any(bm.any() for bm in bad):
            break
        raws2 = _run_cores(per_b, SHIFT0 + delta)
        for b in range(B):
            fixable = bad[b] & ~_bad_cols(raws2[b])
            raws[b][:, fixable] = raws2[b][:, fixable]
            bad[b] &= ~fixable

    nll_sum = 0.0
    with np.errstate(all="ignore"):
        for b in range(B):
            pred = raws[b][:D] / raws[b][D]                  # [16, 4096]
            logp = np.log(pred + EPS)
            idx = labels[b].reshape(T)
            nll_sum += -logp[idx, np.arange(T)].sum()
    loss = nll_sum / (B * T)
    return np.asarray(loss, dtype=np.float32)
